# revision 23
# baseline (speedup 1.0000x reference)
"""Distributed Trainium2 kernel for nn_Attention_29145648070993.

Full-IO contract: kernel(**inputs) takes the complete tensors and returns the
full (out, attn) pair, sharding internally across 8 NeuronCores:
  core c -> (batch b = c//2, head-group hg = c%2 covering 4 of 8 heads).

On-device layout is fully "transposed attention": S^T[m, n] = k @ q^T is
computed per (head, m-tile) so that P^T = exp(S^T) is directly the moving
operand of the attn@v matmul (no on-chip transposes of the big [N, N]
tensor). S^T uses 64x128 PE row-tiling: q'/k' are duplicated into both
partition halves so two m-tiles compute concurrently (K=64 would otherwise
idle half the array). A ones-column appended to v yields softmax row-sums
for free in the attn@v matmul. attn is stored bf16 in [m, n] layout; the
host transposes and upcasts while unsharding. The two w_proj row-block
partials per batch are summed on the host along with b_proj.
"""

import sys

import numpy as np
import ml_dtypes

for _p in ("/opt/trn_rl_repo",):
    try:
        import concourse  # noqa: F401
    except ImportError:
        if _p not in sys.path:
            sys.path.insert(0, _p)

B, N, C, H = 4, 2048, 512, 8
D = C // H            # 64 head dim
HPC = H // 2          # 4 heads per core
DH = HPC * D          # 256 context cols per core
NCORES = 8
ROPE_BASE = 10000.0
BF16 = ml_dtypes.bfloat16

_CACHE = {}


def _rope_tables():
    """cos/sin in [d, n] layout, 2 head-copies stacked to 128 partitions.

    sin is sign-folded so that rope(q) = q*cos + qswap*sin_signed where
    qswap swaps partition blocks [0:32]<->[32:64] within each head.
    """
    inv_freq = 1.0 / (ROPE_BASE ** (np.arange(0, D, 2, dtype=np.float64) / D))  # [32]
    t = np.arange(N, dtype=np.float64)
    freqs = t[:, None] * inv_freq[None, :]              # [N, 32]
    emb = np.concatenate([freqs, freqs], axis=-1)       # [N, 64]
    cos = np.cos(emb).T.astype(np.float32)              # [64, N]
    sin = np.sin(emb).T.astype(np.float32)              # [64, N]
    sin_signed = np.concatenate([-sin[:32], sin[32:]], axis=0)
    cos2 = np.tile(cos, (2, 1)).astype(BF16)            # [128, N]
    sin2 = np.tile(sin_signed, (2, 1)).astype(BF16)     # [128, N]
    return cos2, sin2


def _build():
    import concourse.bass as bass  # noqa: F401
    import concourse.tile as tile
    from concourse import bacc, mybir
    from concourse.masks import make_identity

    f32 = mybir.dt.float32
    bf16 = mybir.dt.bfloat16
    Exp = mybir.ActivationFunctionType.Exp

    nc = bacc.Bacc("TRN2", target_bir_lowering=False, debug=False,
                   num_devices=NCORES)

    x_d = nc.dram_tensor("x", [N, C], bf16, kind="ExternalInput")
    wq_d = nc.dram_tensor("wq", [C, DH], bf16, kind="ExternalInput")
    wk_d = nc.dram_tensor("wk", [C, DH], bf16, kind="ExternalInput")
    wv_d = nc.dram_tensor("wv", [C, DH], bf16, kind="ExternalInput")
    wp_d = nc.dram_tensor("wp", [DH, C], bf16, kind="ExternalInput")
    cos_d = nc.dram_tensor("cos", [128, N], bf16, kind="ExternalInput")
    sin_d = nc.dram_tensor("sin", [128, N], bf16, kind="ExternalInput")
    attn_d = nc.dram_tensor("attn_t", [HPC, N, N], bf16, kind="ExternalOutput")
    y_d = nc.dram_tensor("y", [N, C], f32, kind="ExternalOutput")

    NT = N // 128     # 16 tiles of 128 along n/m
    NS = N // 512     # 4 slices of 512 along n
    NP = NT // 2      # 8 m-tile pairs per head

    with tile.TileContext(nc) as tc:
        with (
            tc.tile_pool(name="persist", bufs=1) as pp,
            tc.tile_pool(name="big", bufs=20) as big,   # [128, 2, N] bf16 slots
            tc.tile_pool(name="work", bufs=2) as wk,
            tc.tile_pool(name="ps", bufs=2, space="PSUM") as ps,
            tc.tile_pool(name="psav", bufs=1, space="PSUM") as psav,
        ):
            ident = pp.tile([128, 128], bf16)
            make_identity(nc, ident[:])

            wq_sb = pp.tile([128, 4, DH], bf16)
            wk_sb = pp.tile([128, 4, DH], bf16)
            wv_sb = pp.tile([128, 4, DH], bf16)
            for cc in range(4):
                nc.gpsimd.dma_start(wq_sb[:, cc, :], wq_d[cc * 128:(cc + 1) * 128, :])
                nc.gpsimd.dma_start(wk_sb[:, cc, :], wk_d[cc * 128:(cc + 1) * 128, :])
                nc.gpsimd.dma_start(wv_sb[:, cc, :], wv_d[cc * 128:(cc + 1) * 128, :])
            wp_sb = pp.tile([128, 2, C], bf16)
            for cc in range(2):
                nc.gpsimd.dma_start(wp_sb[:, cc, :], wp_d[cc * 128:(cc + 1) * 128, :])
            cs_sb = big.tile([128, 2, N], bf16, tag="pt", name="cs_sb")
            nc.gpsimd.dma_start(cs_sb[:, 0, :], cos_d[:])
            nc.gpsimd.dma_start(cs_sb[:, 1, :], sin_d[:])

            # ---- x^T via DMA-transpose on both HWDGE queues ----
            xtp = [big.tile([128, 2, N], bf16, tag="pt", name=f"xtp{i}")
                   for i in range(2)]

            def xts(cc):
                return xtp[cc // 2][:, cc % 2, :]

            for half in range(2):
                for cc in range(4):
                    eng = nc.sync if cc % 2 == 0 else nc.scalar
                    eng.dma_start_transpose(
                        xts(cc)[:, half * 1024:(half + 1) * 1024],
                        x_d[half * 1024:(half + 1) * 1024,
                            cc * 128:(cc + 1) * 128])

            v_ext = pp.tile([128, NT, HPC, D + 1], bf16)
            nc.gpsimd.memset(v_ext[:], 1.0)

            def v_round(mt):
                v_ps = ps.tile([128, DH], f32, tag="ps", name="v_ps")
                for cc in range(4):
                    nc.tensor.matmul(
                        v_ps[:], xts(cc)[:, mt * 128:(mt + 1) * 128],
                        wv_sb[:, cc, :], start=cc == 0, stop=cc == 3)
                for h in range(HPC):
                    nc.scalar.copy(v_ext[:, mt, h, 0:D],
                                   v_ps[:, h * D:(h + 1) * D])

            # logical qk tiles: t=0 q(h01), t=1 q(h23), t=2 k(h01), t=3 k(h23)
            qkp = [big.tile([128, 2, N], bf16, tag="pt", name=f"qkp{i}")
                   for i in range(2)]

            def qk(t):
                return qkp[t // 2][:, t % 2, :]

            def qkv_round(t, ns):
                wsb = wq_sb if t < 2 else wk_sb
                csl = slice((t % 2) * 128, (t % 2) * 128 + 128)
                qk_ps = ps.tile([128, 512], f32, tag="ps", name="qk_ps")
                for cc in range(4):
                    nc.tensor.matmul(
                        qk_ps[:], wsb[:, cc, csl],
                        xts(cc)[:, ns * 512:(ns + 1) * 512],
                        start=cc == 0, stop=cc == 3)
                nc.scalar.copy(qk(t)[:, ns * 512:(ns + 1) * 512], qk_ps[:])

            dup = [big.tile([128, 2, N], bf16, tag="pt", name=f"dup{h}")
                   for h in range(HPC)]

            def rope(t):
                qs = big.tile([128, 2, N], bf16, tag="pt", name="qs")
                for g in range(4):
                    srcg = 32 * (g ^ 1)
                    nc.vector.tensor_copy(qs[32 * g:32 * g + 32, 0, :],
                                          qk(t)[srcg:srcg + 32, :])
                nc.vector.tensor_mul(qk(t)[:, :], qk(t)[:, :], cs_sb[:, 0, :])
                nc.vector.tensor_mul(qs[:, 0, :], qs[:, 0, :], cs_sb[:, 1, :])
                nc.vector.tensor_add(qk(t)[:, :], qk(t)[:, :], qs[:, 0, :])

            def dup_fill(h):
                prow = slice((h % 2) * 64, (h % 2) * 64 + 64)
                for half in range(2):
                    hrow = slice(half * 64, half * 64 + 64)
                    nc.vector.tensor_copy(dup[h][hrow, 0, :], qk(h // 2)[prow, :])
                    nc.vector.tensor_copy(dup[h][hrow, 1, :],
                                          qk(2 + h // 2)[prow, :])

            # dense N=512 qkv streams first (HAM warm-up), v rounds after;
            # v(8-15) refills the PE while the rope/dup DVE chain runs
            for t in (0, 2, 1, 3):
                for ns in (0, 1):
                    qkv_round(t, ns)
            for mt in range(8):
                v_round(mt)
            for t in (0, 2, 1, 3):
                for ns in (2, 3):
                    qkv_round(t, ns)
            rope(0), rope(2)
            dup_fill(0), dup_fill(1)
            for mt in range(8, NT):
                v_round(mt)
            rope(1), rope(3)
            dup_fill(2), dup_fill(3)

            # ---- attention per head; m-tiles processed in row-tiled pairs ----
            ctx_sb = pp.tile([128, 2, N], bf16)      # normalized ctx^T
            scale = float(D) ** -0.5
            for h in range(HPC):
                av_ps = psav.tile([65, N], f32, tag="av", name="av_ps")
                prs = []
                steps = [(p, s) for p in range(NP) for s in range(NS)]

                def av_mms(p_, s_):
                    ssl_ = slice(s_ * 512, (s_ + 1) * 512)
                    for j in range(2):
                        nc.tensor.matmul(
                            av_ps[:, ssl_], v_ext[:, 2 * p_ + j, h, :],
                            prs[p_][:, j, ssl_],
                            start=(p_ == 0 and j == 0),
                            stop=(p_ == NP - 1 and j == 1))

                for i, (p, s) in enumerate(steps):
                    if s == 0:
                        pr = big.tile([128, 2, N], bf16, tag="pt", name="pr")
                        prs.append(pr)
                    mt0, mt1 = 2 * p, 2 * p + 1
                    st_ps = ps.tile([128, 1024], f32, tag="ps", name="st_ps")
                    ssl = slice(s * 512, (s + 1) * 512)
                    nc.tensor.matmul(
                        st_ps[:, 0:512],
                        dup[h][0:64, 1, mt0 * 128:(mt0 + 1) * 128],
                        dup[h][0:64, 0, ssl],
                        start=True, stop=True, tile_position=(0, 0))
                    nc.tensor.matmul(
                        st_ps[:, 512:1024],
                        dup[h][64:128, 1, mt1 * 128:(mt1 + 1) * 128],
                        dup[h][64:128, 0, ssl],
                        start=True, stop=True, tile_position=(64, 0))
                    nc.scalar.activation(prs[p][:, :, ssl], st_ps[:],
                                         Exp, scale=scale)
                    if i >= 2:
                        av_mms(*steps[i - 2])
                av_mms(*steps[-2])
                av_mms(*steps[-1])

                # evacuate av psum quickly so the next head can start
                avc = wk.tile([65, N], f32, tag="avc", name="avc", bufs=1)
                nc.scalar.copy(avc[:], av_ps[:])

                rb16 = wk.tile([128, N], bf16, tag="rb16", name="rb16")
                r128 = wk.tile([128, N // 128], f32, tag="r128", name="r128")
                r16b = wk.tile([128, N // 128], bf16, tag="r16b", name="r16b")
                nc.scalar.dma_start(r128[:], avc[64:65, :])
                nc.vector.reciprocal(r128[:], r128[:])
                nc.vector.tensor_copy(r16b[:], r128[:])
                nc.scalar.dma_start(rb16[0:1, :], r16b[:])
                nc.gpsimd.partition_broadcast(rb16[:], rb16[0:1, :])

                # normalized context^T (bf16) for the projection
                prow = slice((h % 2) * 64, (h % 2) * 64 + 64)
                nc.vector.tensor_mul(ctx_sb[prow, h // 2, :], avc[0:64, :],
                                     rb16[0:64, :])

                # normalize P^T pair tiles in place and store (1MB per DMA)
                for p in range(NP):
                    pr = prs[p]
                    for j in range(2):
                        nc.vector.tensor_mul(pr[:, j, :], pr[:, j, :], rb16[:])
                    deng = nc.gpsimd if p % 4 == 1 else nc.sync
                    deng.dma_start(
                        attn_d[h, p * 256:(p + 1) * 256, :]
                        .rearrange("(j p_) n -> p_ j n", j=2), pr[:])

                # projection can start as soon as the last head's ctx lands
                if h == HPC - 1:
                    for nt in range(NT):
                        y_ps = ps.tile([128, C], f32, tag="ps", name="y_ps")
                        for cc in range(2):
                            nc.tensor.matmul(
                                y_ps[:], ctx_sb[:, cc, nt * 128:(nt + 1) * 128],
                                wp_sb[:, cc, :], start=cc == 0, stop=cc == 1)
                        y_sb = wk.tile([128, C], f32, tag="ysb", name="y_sb", bufs=3)
                        nc.scalar.copy(y_sb[:], y_ps[:])
                        nc.sync.dma_start(y_d[nt * 128:(nt + 1) * 128, :],
                                            y_sb[:])

    nc.compile()
    return nc


def _get_nc():
    if "nc" not in _CACHE:
        _CACHE["nc"] = _build()
    return _CACHE["nc"]


def _make_in_maps(x, w_qkv, w_proj):
    cos2, sin2 = _rope_tables()
    x = np.asarray(x)
    w_qkv = np.asarray(w_qkv)
    w_proj = np.asarray(w_proj)
    in_maps = []
    for c in range(NCORES):
        b, hg = c // 2, c % 2
        cols = slice(hg * DH, (hg + 1) * DH)
        in_maps.append({
            "x": np.ascontiguousarray(x[b]).astype(BF16),
            "wq": np.ascontiguousarray(w_qkv[:, 0 * C:1 * C][:, cols]).astype(BF16),
            "wk": np.ascontiguousarray(w_qkv[:, 1 * C:2 * C][:, cols]).astype(BF16),
            "wv": np.ascontiguousarray(w_qkv[:, 2 * C:3 * C][:, cols]).astype(BF16),
            "wp": np.ascontiguousarray(w_proj[hg * DH:(hg + 1) * DH, :]).astype(BF16),
            "cos": cos2,
            "sin": sin2,
        })
    return in_maps


def _run(inputs, trace=False, trace_kwargs=None):
    from concourse.bass_utils import run_bass_kernel_spmd
    nc = _get_nc()
    in_maps = _make_in_maps(inputs["x"], inputs["w_qkv"], inputs["w_proj"])
    res = run_bass_kernel_spmd(nc, in_maps, core_ids=list(range(NCORES)),
                               trace=trace, **(trace_kwargs or {}))
    return res


def _assemble(results, b_proj):
    attn = np.empty((B, H, N, N), dtype=np.float32)
    out = np.empty((B, N, C), dtype=np.float32)
    for c in range(NCORES):
        b, hg = c // 2, c % 2
        at = np.asarray(results[c]["attn_t"]).astype(np.float32)  # [HPC, m, n]
        attn[b, hg * HPC:(hg + 1) * HPC] = at.transpose(0, 2, 1)
    bp = np.asarray(b_proj, dtype=np.float32)
    for b in range(B):
        out[b] = (np.asarray(results[2 * b]["y"], dtype=np.float32)
                  + np.asarray(results[2 * b + 1]["y"], dtype=np.float32) + bp)
    return out, attn


def kernel(x, w_qkv, w_proj, b_proj):
    res = _run({"x": x, "w_qkv": w_qkv, "w_proj": w_proj})
    return _assemble(res.results, b_proj)


# revision 24
# speedup vs baseline: 1.0553x; 1.0553x over previous
"""Distributed Trainium2 kernel for nn_Attention_29145648070993.

Full-IO contract: kernel(**inputs) takes the complete tensors and returns the
full (out, attn) pair, sharding internally across 8 NeuronCores:
  core c -> (batch b = c//2, head-group hg = c%2 covering 4 of 8 heads).

On-device layout is fully "transposed attention": S^T[m, n] = k @ q^T is
computed per (head, m-tile) so that P^T = exp(S^T) is directly the moving
operand of the attn@v matmul (no on-chip transposes of the big [N, N]
tensor). S^T uses 64x128 PE row-tiling: q'/k' are duplicated into both
partition halves so two m-tiles compute concurrently (K=64 would otherwise
idle half the array). A ones-column appended to v yields softmax row-sums
for free in the attn@v matmul. attn is stored bf16 in [m, n] layout; the
host transposes and upcasts while unsharding. The two w_proj row-block
partials per batch are summed on the host along with b_proj.
"""

import sys

import numpy as np
import ml_dtypes

for _p in ("/opt/trn_rl_repo",):
    try:
        import concourse  # noqa: F401
    except ImportError:
        if _p not in sys.path:
            sys.path.insert(0, _p)

B, N, C, H = 4, 2048, 512, 8
D = C // H            # 64 head dim
HPC = H // 2          # 4 heads per core
DH = HPC * D          # 256 context cols per core
NCORES = 8
ROPE_BASE = 10000.0
BF16 = ml_dtypes.bfloat16

_CACHE = {}


def _rope_tables():
    """cos/sin in [d, n] layout, 2 head-copies stacked to 128 partitions.

    sin is sign-folded so that rope(q) = q*cos + qswap*sin_signed where
    qswap swaps partition blocks [0:32]<->[32:64] within each head.
    """
    inv_freq = 1.0 / (ROPE_BASE ** (np.arange(0, D, 2, dtype=np.float64) / D))  # [32]
    t = np.arange(N, dtype=np.float64)
    freqs = t[:, None] * inv_freq[None, :]              # [N, 32]
    emb = np.concatenate([freqs, freqs], axis=-1)       # [N, 64]
    cos = np.cos(emb).T.astype(np.float32)              # [64, N]
    sin = np.sin(emb).T.astype(np.float32)              # [64, N]
    sin_signed = np.concatenate([-sin[:32], sin[32:]], axis=0)
    cos2 = np.tile(cos, (2, 1)).astype(BF16)            # [128, N]
    sin2 = np.tile(sin_signed, (2, 1)).astype(BF16)     # [128, N]
    return cos2, sin2


def _build():
    import concourse.bass as bass  # noqa: F401
    import concourse.tile as tile
    from concourse import bacc, mybir
    from concourse.masks import make_identity

    f32 = mybir.dt.float32
    bf16 = mybir.dt.bfloat16
    Exp = mybir.ActivationFunctionType.Exp

    nc = bacc.Bacc("TRN2", target_bir_lowering=False, debug=False,
                   num_devices=NCORES)

    x_d = nc.dram_tensor("x", [N, C], bf16, kind="ExternalInput")
    wq_d = nc.dram_tensor("wq", [C, DH], bf16, kind="ExternalInput")
    wk_d = nc.dram_tensor("wk", [C, DH], bf16, kind="ExternalInput")
    wv_d = nc.dram_tensor("wv", [C, DH], bf16, kind="ExternalInput")
    wp_d = nc.dram_tensor("wp", [DH, C], bf16, kind="ExternalInput")
    cos_d = nc.dram_tensor("cos", [128, N], bf16, kind="ExternalInput")
    sin_d = nc.dram_tensor("sin", [128, N], bf16, kind="ExternalInput")
    attn_d = nc.dram_tensor("attn_t", [HPC, N, N], bf16, kind="ExternalOutput")
    y_d = nc.dram_tensor("y", [N, C], f32, kind="ExternalOutput")

    NT = N // 128     # 16 tiles of 128 along n/m
    NS = N // 512     # 4 slices of 512 along n
    NP = NT // 2      # 8 m-tile pairs per head

    with tile.TileContext(nc) as tc:
        with (
            tc.tile_pool(name="persist", bufs=1) as pp,
            tc.tile_pool(name="big", bufs=19) as big,   # [128, 2, N] bf16 slots
            tc.tile_pool(name="work", bufs=2) as wk,
            tc.tile_pool(name="ps", bufs=2, space="PSUM") as ps,
            tc.tile_pool(name="psav", bufs=1, space="PSUM") as psav,
        ):
            ident = pp.tile([128, 128], bf16)
            make_identity(nc, ident[:])

            wq_sb = pp.tile([128, 4, DH], bf16)
            wk_sb = pp.tile([128, 4, DH], bf16)
            wv_sb = pp.tile([128, 4, DH], bf16)
            for cc in range(4):
                nc.gpsimd.dma_start(wq_sb[:, cc, :], wq_d[cc * 128:(cc + 1) * 128, :])
                nc.gpsimd.dma_start(wk_sb[:, cc, :], wk_d[cc * 128:(cc + 1) * 128, :])
                nc.gpsimd.dma_start(wv_sb[:, cc, :], wv_d[cc * 128:(cc + 1) * 128, :])
            wp_sb = pp.tile([128, 2, C], bf16)
            for cc in range(2):
                nc.gpsimd.dma_start(wp_sb[:, cc, :], wp_d[cc * 128:(cc + 1) * 128, :])
            cs_sb = big.tile([128, 2, N], bf16, tag="pt", name="cs_sb")
            nc.gpsimd.dma_start(cs_sb[:, 0, :], cos_d[:])
            nc.gpsimd.dma_start(cs_sb[:, 1, :], sin_d[:])

            # ---- x^T via DMA-transpose on both HWDGE queues ----
            xtp = [big.tile([128, 2, N], bf16, tag="pt", name=f"xtp{i}")
                   for i in range(2)]

            def xts(cc):
                return xtp[cc // 2][:, cc % 2, :]

            for half in range(2):
                for cc in range(4):
                    eng = nc.sync if cc % 2 == 0 else nc.scalar
                    eng.dma_start_transpose(
                        xts(cc)[:, half * 1024:(half + 1) * 1024],
                        x_d[half * 1024:(half + 1) * 1024,
                            cc * 128:(cc + 1) * 128])

            v_ext = pp.tile([128, NT, HPC, D + 1], bf16)
            nc.gpsimd.memset(v_ext[:], 1.0)

            def v_round(mt):
                v_ps = ps.tile([128, DH], f32, tag="ps", name="v_ps")
                for cc in range(4):
                    nc.tensor.matmul(
                        v_ps[:], xts(cc)[:, mt * 128:(mt + 1) * 128],
                        wv_sb[:, cc, :], start=cc == 0, stop=cc == 3)
                for h in range(HPC):
                    nc.scalar.copy(v_ext[:, mt, h, 0:D],
                                   v_ps[:, h * D:(h + 1) * D])

            # logical qk tiles: t=0 q(h01), t=1 q(h23), t=2 k(h01), t=3 k(h23)
            qkp = [big.tile([128, 2, N], bf16, tag="pt", name=f"qkp{i}")
                   for i in range(2)]

            def qk(t):
                return qkp[t // 2][:, t % 2, :]

            def qkv_round(t, ns):
                wsb = wq_sb if t < 2 else wk_sb
                csl = slice((t % 2) * 128, (t % 2) * 128 + 128)
                qk_ps = ps.tile([128, 512], f32, tag="ps", name="qk_ps")
                for cc in range(4):
                    nc.tensor.matmul(
                        qk_ps[:], wsb[:, cc, csl],
                        xts(cc)[:, ns * 512:(ns + 1) * 512],
                        start=cc == 0, stop=cc == 3)
                nc.scalar.copy(qk(t)[:, ns * 512:(ns + 1) * 512], qk_ps[:])

            dup = [big.tile([128, 2, N], bf16, tag="pt", name=f"dup{h}")
                   for h in range(HPC)]

            def rope(t):
                qs = big.tile([128, 2, N], bf16, tag="pt", name="qs")
                for g in range(4):
                    srcg = 32 * (g ^ 1)
                    nc.vector.tensor_copy(qs[32 * g:32 * g + 32, 0, :],
                                          qk(t)[srcg:srcg + 32, :])
                nc.vector.tensor_mul(qk(t)[:, :], qk(t)[:, :], cs_sb[:, 0, :])
                nc.vector.tensor_mul(qs[:, 0, :], qs[:, 0, :], cs_sb[:, 1, :])
                nc.vector.tensor_add(qk(t)[:, :], qk(t)[:, :], qs[:, 0, :])

            def dup_fill(h):
                prow = slice((h % 2) * 64, (h % 2) * 64 + 64)
                for half in range(2):
                    hrow = slice(half * 64, half * 64 + 64)
                    nc.vector.tensor_copy(dup[h][hrow, 0, :], qk(h // 2)[prow, :])
                    nc.vector.tensor_copy(dup[h][hrow, 1, :],
                                          qk(2 + h // 2)[prow, :])

            # dense N=512 qkv streams first (HAM warm-up), v rounds after;
            # v(8-15) refills the PE while the rope/dup DVE chain runs
            for t in (0, 2, 1, 3):
                for ns in (0, 1):
                    qkv_round(t, ns)
            for mt in range(8):
                v_round(mt)
            for t in (0, 2, 1, 3):
                for ns in (2, 3):
                    qkv_round(t, ns)
            rope(0), rope(2)
            dup_fill(0), dup_fill(1)
            for mt in range(8, NT):
                v_round(mt)
            rope(1), rope(3)
            dup_fill(2), dup_fill(3)

            # ---- attention per head; m-tiles processed in row-tiled pairs ----
            ctx_sb = pp.tile([128, 2, N], bf16)      # normalized ctx^T
            scale = float(D) ** -0.5
            for h in range(HPC):
                av_ps = psav.tile([65, N], f32, tag="av", name="av_ps")
                prs = []
                steps = [(p, s) for p in range(NP) for s in range(NS)]

                def av_mms(p_, s_):
                    ssl_ = slice(s_ * 512, (s_ + 1) * 512)
                    for j in range(2):
                        nc.tensor.matmul(
                            av_ps[:, ssl_], v_ext[:, 2 * p_ + j, h, :],
                            prs[p_][:, j, ssl_],
                            start=(p_ == 0 and j == 0),
                            stop=(p_ == NP - 1 and j == 1))

                for i, (p, s) in enumerate(steps):
                    if s == 0:
                        pr = big.tile([128, 2, N], bf16, tag="pt", name="pr")
                        prs.append(pr)
                    mt0, mt1 = 2 * p, 2 * p + 1
                    st_ps = ps.tile([128, 1024], f32, tag="ps", name="st_ps")
                    ssl = slice(s * 512, (s + 1) * 512)
                    nc.tensor.matmul(
                        st_ps[:, 0:512],
                        dup[h][0:64, 1, mt0 * 128:(mt0 + 1) * 128],
                        dup[h][0:64, 0, ssl],
                        start=True, stop=True, tile_position=(0, 0))
                    nc.tensor.matmul(
                        st_ps[:, 512:1024],
                        dup[h][64:128, 1, mt1 * 128:(mt1 + 1) * 128],
                        dup[h][64:128, 0, ssl],
                        start=True, stop=True, tile_position=(64, 0))
                    nc.scalar.activation(prs[p][:, :, ssl], st_ps[:],
                                         Exp, scale=scale)
                    if i >= 2:
                        av_mms(*steps[i - 2])
                av_mms(*steps[-2])
                av_mms(*steps[-1])

                # evacuate av psum quickly so the next head can start
                avc = wk.tile([65, N], f32, tag="avc", name="avc", bufs=1)
                nc.scalar.copy(avc[:], av_ps[:])

                rb16 = wk.tile([128, N], bf16, tag="rb16", name="rb16")
                r128 = wk.tile([128, N // 128], f32, tag="r128", name="r128")
                r16b = wk.tile([128, N // 128], bf16, tag="r16b", name="r16b")
                nc.scalar.dma_start(r128[:], avc[64:65, :])
                nc.vector.reciprocal(r128[:], r128[:])
                nc.vector.tensor_copy(r16b[:], r128[:])
                nc.scalar.dma_start(rb16[0:1, :], r16b[:])
                nc.gpsimd.partition_broadcast(rb16[:], rb16[0:1, :])

                # normalized context^T (bf16) for the projection
                prow = slice((h % 2) * 64, (h % 2) * 64 + 64)
                nc.vector.tensor_mul(ctx_sb[prow, h // 2, :], avc[0:64, :],
                                     rb16[0:64, :])

                # normalize P^T pair tiles in place and store (1MB per DMA)
                for p in range(NP):
                    pr = prs[p]
                    for j in range(2):
                        nc.vector.tensor_mul(pr[:, j, :], pr[:, j, :], rb16[:])
                    deng = nc.gpsimd if p % 4 == 1 else nc.sync
                    deng.dma_start(
                        attn_d[h, p * 256:(p + 1) * 256, :]
                        .rearrange("(j p_) n -> p_ j n", j=2), pr[:])

                # projection can start as soon as the last head's ctx lands
                if h == HPC - 1:
                    for nt in range(NT):
                        y_ps = ps.tile([128, C], f32, tag="ps", name="y_ps")
                        for cc in range(2):
                            nc.tensor.matmul(
                                y_ps[:], ctx_sb[:, cc, nt * 128:(nt + 1) * 128],
                                wp_sb[:, cc, :], start=cc == 0, stop=cc == 1)
                        y_sb = wk.tile([128, C], f32, tag="ysb", name="y_sb", bufs=4)
                        nc.scalar.copy(y_sb[:], y_ps[:])
                        nc.sync.dma_start(y_d[nt * 128:(nt + 1) * 128, :],
                                            y_sb[:])

    nc.compile()
    return nc


def _get_nc():
    if "nc" not in _CACHE:
        _CACHE["nc"] = _build()
    return _CACHE["nc"]


def _make_in_maps(x, w_qkv, w_proj):
    cos2, sin2 = _rope_tables()
    x = np.asarray(x)
    w_qkv = np.asarray(w_qkv)
    w_proj = np.asarray(w_proj)
    in_maps = []
    for c in range(NCORES):
        b, hg = c // 2, c % 2
        cols = slice(hg * DH, (hg + 1) * DH)
        in_maps.append({
            "x": np.ascontiguousarray(x[b]).astype(BF16),
            "wq": np.ascontiguousarray(w_qkv[:, 0 * C:1 * C][:, cols]).astype(BF16),
            "wk": np.ascontiguousarray(w_qkv[:, 1 * C:2 * C][:, cols]).astype(BF16),
            "wv": np.ascontiguousarray(w_qkv[:, 2 * C:3 * C][:, cols]).astype(BF16),
            "wp": np.ascontiguousarray(w_proj[hg * DH:(hg + 1) * DH, :]).astype(BF16),
            "cos": cos2,
            "sin": sin2,
        })
    return in_maps


def _run(inputs, trace=False, trace_kwargs=None):
    from concourse.bass_utils import run_bass_kernel_spmd
    nc = _get_nc()
    in_maps = _make_in_maps(inputs["x"], inputs["w_qkv"], inputs["w_proj"])
    res = run_bass_kernel_spmd(nc, in_maps, core_ids=list(range(NCORES)),
                               trace=trace, **(trace_kwargs or {}))
    return res


def _assemble(results, b_proj):
    attn = np.empty((B, H, N, N), dtype=np.float32)
    out = np.empty((B, N, C), dtype=np.float32)
    for c in range(NCORES):
        b, hg = c // 2, c % 2
        at = np.asarray(results[c]["attn_t"]).astype(np.float32)  # [HPC, m, n]
        attn[b, hg * HPC:(hg + 1) * HPC] = at.transpose(0, 2, 1)
    bp = np.asarray(b_proj, dtype=np.float32)
    for b in range(B):
        out[b] = (np.asarray(results[2 * b]["y"], dtype=np.float32)
                  + np.asarray(results[2 * b + 1]["y"], dtype=np.float32) + bp)
    return out, attn


def kernel(x, w_qkv, w_proj, b_proj):
    res = _run({"x": x, "w_qkv": w_qkv, "w_proj": w_proj})
    return _assemble(res.results, b_proj)
